# revision 17
# baseline (speedup 1.0000x reference)
"""Trainium2 Bass kernel for nn_LongTermMemoryMLP.

Per-batch-weight 3-layer MLP:
    h0 = relu(q @ W0^T + b0); h1 = relu(h0 @ W1^T + b1); out = h1 @ W2^T + b2
with q: [B,S,DIN], W0: [B,DH,DIN], W1: [B,DH,DH], W2: [B,DOUT,DH], B=8.

Sharding: data-parallel over batch — one batch sample (and its weight slabs)
per NeuronCore, 8 cores, no cross-core communication.

Device-side strategy (v2): everything feature-major. Activations live as
[feature, seq] tiles (feature on partitions), weights are pre-transposed on
the host, so every layer is stationary=weight-slice, moving=activation —
including layer 2, whose output lands transposed ([DOUT, S]) and is
un-transposed on the host. That makes every bias a per-partition scalar,
applied for free in the scalar-engine activation that drains PSUM (no
broadcast-bias DMA, vector engine freed up for stores).

All matmul operands are bf16 (tolerance is 2e-2; measured pipeline error
~4.3e-3): halves HBM traffic vs fp32r (startup-critical) and enables the
fast-weight-load path on LDWEIGHTS. PSUM accumulation stays fp32.

Startup is the other big lever (the fixed engine preamble ends ~6.5us, and
the PE must be streaming real matmuls as soon after as possible): the
layer-0 weights and first seq chunk are spread across all four DMA queues
(sync/scalar HWDGE, gpsimd/vector SWDGE) so the first k-groups' operands
land ~2.5us after the rings go live, with a short burst of dummy bf16
matmuls keeping the PE-HAM clock gate warm until they do.
"""

import numpy as np

import ml_dtypes

import concourse.bass as bass
import concourse.tile as tile
from concourse import bacc, mybir
from concourse.bass_utils import run_bass_kernel_spmd

B, S, DIN, DH, DOUT = 8, 4096, 512, 1024, 512
SC = 512  # seq chunk processed per pipeline iteration

BF16 = mybir.dt.bfloat16
F32 = mybir.dt.float32

K0 = DIN // 128   # 4  k-tiles, layer 0
K1 = DH // 128    # 8  k-tiles, layers 1/2
M0 = DH // 128    # 8  m-tiles (feature tiles of h0/h1)
M2 = DOUT // 128  # 4  m-tiles (feature tiles of outT)
NCH = S // SC     # 8  chunks

N_WARM = 26      # dummy bf16 matmuls bridging preamble-end -> first data


def build_nc():
    nc = bacc.Bacc("TRN2")
    qT = nc.dram_tensor("qT", (DIN, S), BF16, kind="ExternalInput")
    w0t = nc.dram_tensor("w0t", (DIN, DH), BF16, kind="ExternalInput")
    w1t = nc.dram_tensor("w1t", (DH, DH), BF16, kind="ExternalInput")
    w2t = nc.dram_tensor("w2t", (DH, DOUT), BF16, kind="ExternalInput")
    # b0 | b1 | b2, host-packed as [128 partitions, 8+8+4 cols] so the load
    # is one contiguous-line DMA (the on-device scatter rearrange was taking
    # ~10us on the wire and stalling the ring's semaphore rotation).
    biases = nc.dram_tensor("biases", (128, 2 * M0 + M2), F32, kind="ExternalInput")
    outT = nc.dram_tensor("outT", (DOUT, S), F32, kind="ExternalOutput")

    Relu = mybir.ActivationFunctionType.Relu
    Ident = mybir.ActivationFunctionType.Identity

    with tile.TileContext(nc) as tc:
        with (
            tc.tile_pool(name="weights", bufs=1) as wpool,
            tc.tile_pool(name="biases", bufs=1) as bpool,
            tc.tile_pool(name="acts", bufs=2) as apool,
            tc.tile_pool(name="qin", bufs=3) as qpool,
            tc.tile_pool(name="outp", bufs=6) as opool,
            tc.tile_pool(name="psum0", bufs=3, space="PSUM") as ppool0,
            tc.tile_pool(name="psum1", bufs=3, space="PSUM") as ppool1,
            tc.tile_pool(name="psum2", bufs=2, space="PSUM") as ppool2,
        ):
            # Tiny warm tiles for the HAM warm-up matmuls.
            g_lhs = apool.tile([128, 128], BF16, tag="warm_lhs")
            g_rhs = apool.tile([128, 256], BF16, tag="warm_rhs")
            nc.vector.memset(g_lhs, 0.0)
            nc.vector.memset(g_rhs, 0.0)

            # ---- startup loads ----
            # The three DMA queues (sync/scalar HWDGE + gpsimd SWDGE) share
            # the ~358 GB/s HBM pipe (~110-120 GB/s each while all are
            # busy), and each is FIFO — so per-queue issue order is arranged
            # to make tiles land exactly in consumption order: the k-groups
            # of (w0, q0) first (w0 k-tiles split in half across both HWDGE
            # rings), then q1, then W1, then W2, then the q stream.
            w0_sb = [wpool.tile([128, DH], BF16, tag=f"w0_{k}", name=f"w0_{k}") for k in range(K0)]
            q0_sb = [qpool.tile([128, SC], BF16, tag=f"q_{k}", name=f"q0_{k}") for k in range(K0)]

            def packed_ktiles(t, row0, nk, ncols):
                # AP reading nk consecutive 128-row k-tiles of t, laid out
                # side by side on the 128 partitions: [p][k][col].
                base = t[row0:row0 + 128, 0:ncols]
                ap = [list(dim) for dim in base.ap]
                ap3 = [ap[0], [128 * ap[0][0], nk], ap[1]]
                return bass.AP(tensor=base.tensor, offset=base.offset, ap=ap3)

            def packed_qchunk(s0):
                base = qT[0:128, s0:s0 + SC]
                ap = [list(dim) for dim in base.ap]
                ap3 = [ap[0], [128 * ap[0][0], K0], ap[1]]
                return bass.AP(tensor=base.tensor, offset=base.offset, ap=ap3)

            # w0/q0 stay as per-k-tile pieces so chunk-0's k-outer layer 0
            # can start on k=0 while k=3 is still in flight. The scalar
            # engine gets ONLY these four small loads: any further issues
            # would block on the 4-deep DMA-semaphore rotation and stall
            # the relu/activation stream queued behind them.
            nc.sync.dma_start(out=q0_sb[0], in_=qT[0:128, 0:SC])
            bias_sb = bpool.tile([128, 2 * M0 + M2], F32, tag="biases")
            nc.sync.dma_start(out=bias_sb, in_=biases[:, :])

            for k in range(K0):
                if k > 0:
                    nc.gpsimd.dma_start(out=q0_sb[k], in_=qT[k * 128:(k + 1) * 128, 0:SC])
                nc.sync.dma_start(out=w0_sb[k][:, 0:DH // 2],
                                  in_=w0t[k * 128:(k + 1) * 128, 0:DH // 2])
                nc.scalar.dma_start(out=w0_sb[k][:, DH // 2:DH],
                                    in_=w0t[k * 128:(k + 1) * 128, DH // 2:DH])

            # Second q chunk as k-tile pieces, placed immediately after the
            # chunk-0-critical loads in each ring's FIFO (it is consumed
            # next).
            q1_sb = [qpool.tile([128, SC], BF16, tag=f"q_{k}", name=f"q1pre_{k}")
                     for k in range(K0)]
            nc.sync.dma_start(out=q1_sb[0], in_=qT[0:128, SC:2 * SC])
            nc.gpsimd.dma_start(out=q1_sb[1], in_=qT[128:256, SC:2 * SC])
            nc.gpsimd.dma_start(out=q1_sb[2], in_=qT[256:384, SC:2 * SC])
            nc.gpsimd.dma_start(out=q1_sb[3], in_=qT[384:512, SC:2 * SC])
            # W1 per-k-tile alternating sync/gpsimd so both rings finish
            # their share just before layer 1 of chunk 0 needs it (~23us);
            # W2 follows the same split. Nothing beyond w0 goes on scalar:
            # the Tile scheduler orders scalar's queue by consumer priority
            # and would push extra DMA issues behind the relu stream,
            # starving the ring.
            w1_sb = [wpool.tile([128, DH], BF16, tag=f"w1_{k}", name=f"w1_{k}") for k in range(K1)]
            w2_sb = [wpool.tile([128, DOUT], BF16, tag=f"w2_{k}", name=f"w2_{k}") for k in range(K1)]
            for k in range(K1):
                eng = nc.sync if k % 2 == 0 else nc.gpsimd
                eng.dma_start(out=w1_sb[k], in_=w1t[k * 128:(k + 1) * 128, :])
            for k in range(K1):
                eng = nc.sync if k % 2 == 0 else nc.gpsimd
                eng.dma_start(out=w2_sb[k], in_=w2t[k * 128:(k + 1) * 128, :])

            def w1_slice(k, m):
                return w1_sb[k][:, m * 128:(m + 1) * 128]

            def w2_slice(k, md):
                return w2_sb[k][:, md * 128:(md + 1) * 128]

            # ---- HAM warm-up: keep PE busy from preamble-end until the
            # first real operands land (dummy matmuls, garbage data).
            warm_ps = ppool0.tile([128, SC], F32, tag="ps0")
            for i in range(N_WARM):
                nc.tensor.matmul(
                    warm_ps[:, 0:256], lhsT=g_lhs, rhs=g_rhs,
                    start=(i == 0), stop=(i == N_WARM - 1),
                )

            def layer0_init(q_sb):
                """Chunk-0 layer 0, k-outer: all 8 m-tiles accumulate in
                parallel across all 8 PSUM banks, so the PE consumes
                (w0[k], q0[k]) in exactly the order the DMAs deliver them —
                no m-loop ever waits on a k-tile that hasn't landed."""
                pools = [ppool0, ppool0, ppool0, ppool1, ppool1, ppool1, ppool2, ppool2]
                tags = ["ps0", "ps0", "ps0", "ps1", "ps1", "ps1", "ps2", "ps2"]
                ps = [pools[m].tile([128, SC], F32, tag=tags[m], name=f"psI_{m}")
                      for m in range(M0)]
                h0_sb = []
                for k in range(K0):
                    for m in range(M0):
                        nc.tensor.matmul(
                            ps[m],
                            lhsT=w0_sb[k][:, m * 128:(m + 1) * 128],
                            rhs=q_sb[k],
                            start=(k == 0),
                            stop=(k == K0 - 1),
                        )
                        if k == K0 - 1:
                            h = apool.tile([128, SC], BF16, tag=f"h0_{m}",
                                           name=f"h0_0_{m}")
                            if m % 2 == 0:
                                nc.vector.tensor_scalar(
                                    h, ps[m], bias_sb[:, m:m + 1], 0.0,
                                    mybir.AluOpType.add, mybir.AluOpType.max)
                            else:
                                nc.scalar.activation(h, ps[m], Relu,
                                                     bias=bias_sb[:, m:m + 1])
                            h0_sb.append(h)
                return h0_sb

            def load_q(c):
                s0 = c * SC
                t = qpool.tile([128, K0 * SC], BF16, tag="qc", name=f"q{c}_all")
                nc.gpsimd.dma_start(out=t, in_=packed_qchunk(s0))
                return [t[:, k * SC:(k + 1) * SC] for k in range(K0)]

            def layer0(c, q_sb):
                h0_sb = []
                for m in range(M0):
                    ps = ppool0.tile([128, SC], F32, tag="ps0", name=f"ps0_{c}_{m}")
                    for k in range(K0):
                        nc.tensor.matmul(
                            ps,
                            lhsT=w0_sb[k][:, m * 128:(m + 1) * 128],
                            rhs=q_sb[k],
                            start=(k == 0),
                            stop=(k == K0 - 1),
                        )
                    h = apool.tile([128, SC], BF16, tag=f"h0_{m}", name=f"h0_{c}_{m}")
                    if m % 2 == 0:
                        nc.vector.tensor_scalar(
                            h, ps, bias_sb[:, m:m + 1], 0.0,
                            mybir.AluOpType.add, mybir.AluOpType.max)
                    else:
                        nc.scalar.activation(h, ps, Relu, bias=bias_sb[:, m:m + 1])
                    h0_sb.append(h)
                return h0_sb

            def layers12(c, h0_sb):
                s0 = c * SC
                h1_sb = []
                for m in range(M0):
                    ps = ppool1.tile([128, SC], F32, tag="ps1", name=f"ps1_{c}_{m}")
                    for k in range(K1):
                        nc.tensor.matmul(
                            ps,
                            lhsT=w1_slice(k, m),
                            rhs=h0_sb[k],
                            start=(k == 0),
                            stop=(k == K1 - 1),
                        )
                    h = apool.tile([128, SC], BF16, tag=f"h1_{m}", name=f"h1_{c}_{m}")
                    nc.scalar.activation(h, ps, Relu, bias=bias_sb[:, M0 + m:M0 + m + 1])
                    h1_sb.append(h)

                last = c == NCH - 1
                for md in range(M2):
                    ps = ppool2.tile([128, SC], F32, tag="ps2", name=f"ps2_{c}_{md}")
                    for k in range(K1):
                        nc.tensor.matmul(
                            ps,
                            lhsT=w2_slice(k, md),
                            rhs=h1_sb[k],
                            start=(k == 0),
                            stop=(k == K1 - 1),
                        )
                    if not (last and md == M2 - 1):
                        ot = opool.tile([128, SC], F32, tag="ot", name=f"ot_{c}_{md}")
                        nc.vector.tensor_scalar_add(
                            ot, ps, bias_sb[:, 2 * M0 + md:2 * M0 + md + 1])
                        nc.sync.dma_start(
                            out=outT[md * 128:(md + 1) * 128, s0:s0 + SC], in_=ot
                        )
                    else:
                        # Very last output tile: drain in two 256-col
                        # slices, second store on scalar (idle after its
                        # last ACT) — shortens the tail.
                        for sl in range(2):
                            o = opool.tile([128, SC // 2], F32, tag=f"otl_{sl}",
                                           name=f"otl_{c}_{md}_{sl}")
                            c0s = sl * (SC // 2)
                            nc.vector.tensor_scalar_add(
                                o, ps[:, c0s:c0s + SC // 2],
                                bias_sb[:, 2 * M0 + md:2 * M0 + md + 1],
                            )
                            eng = nc.sync if sl == 0 else nc.scalar
                            eng.dma_start(
                                out=outT[md * 128:(md + 1) * 128,
                                         s0 + c0s:s0 + c0s + SC // 2],
                                in_=o,
                            )

            # Software pipeline: emit L0 of chunk c+1 ahead of L1/L2 of
            # chunk c, so the matmul stream never depends on a DMA issued
            # less than a full chunk earlier.
            h0_cur = layer0_init(q0_sb)
            for c in range(NCH):
                h0_next = None
                if c + 1 < NCH:
                    q_sb = q1_sb if c + 1 == 1 else load_q(c + 1)
                    h0_next = layer0(c + 1, q_sb)
                layers12(c, h0_cur)
                h0_cur = h0_next
    nc.finalize()
    return nc


_NC = None


def _get_nc():
    global _NC
    if _NC is None:
        _NC = build_nc()
    return _NC


def make_in_maps(inputs):
    bf16 = ml_dtypes.bfloat16
    q, W0, b0, W1, b1, W2, b2 = (
        inputs["query"], inputs["W0"], inputs["b0"], inputs["W1"],
        inputs["b1"], inputs["W2"], inputs["b2"],
    )
    in_maps = []
    for b in range(B):
        in_maps.append({
            "qT": np.ascontiguousarray(np.asarray(q[b]).T).astype(bf16),
            "w0t": np.ascontiguousarray(np.asarray(W0[b]).T).astype(bf16),
            "w1t": np.ascontiguousarray(np.asarray(W1[b]).T).astype(bf16),
            "w2t": np.ascontiguousarray(np.asarray(W2[b]).T).astype(bf16),
            "biases": np.concatenate([
                np.asarray(b0[b], np.float32).reshape(M0, 128).T,
                np.asarray(b1[b], np.float32).reshape(M0, 128).T,
                np.asarray(b2[b], np.float32).reshape(M2, 128).T,
            ], axis=1).copy(),
        })
    return in_maps


def run(inputs, trace=False):
    nc = _get_nc()
    in_maps = make_in_maps(inputs)
    res = run_bass_kernel_spmd(nc, in_maps, core_ids=list(range(B)), trace=trace)
    out = np.stack(
        [np.asarray(r["outT"], dtype=np.float32).T for r in res.results]
    )
    return np.ascontiguousarray(out), res


def kernel(**inputs) -> np.ndarray:
    out, _ = run(inputs, trace=False)
    return out


# revision 18
# speedup vs baseline: 1.0164x; 1.0164x over previous
"""Trainium2 Bass kernel for nn_LongTermMemoryMLP.

Per-batch-weight 3-layer MLP:
    h0 = relu(q @ W0^T + b0); h1 = relu(h0 @ W1^T + b1); out = h1 @ W2^T + b2
with q: [B,S,DIN], W0: [B,DH,DIN], W1: [B,DH,DH], W2: [B,DOUT,DH], B=8.

Sharding: data-parallel over batch — one batch sample (and its weight slabs)
per NeuronCore, 8 cores, no cross-core communication.

Device-side strategy (v2): everything feature-major. Activations live as
[feature, seq] tiles (feature on partitions), weights are pre-transposed on
the host, so every layer is stationary=weight-slice, moving=activation —
including layer 2, whose output lands transposed ([DOUT, S]) and is
un-transposed on the host. That makes every bias a per-partition scalar,
applied for free in the scalar-engine activation that drains PSUM (no
broadcast-bias DMA, vector engine freed up for stores).

All matmul operands are bf16 (tolerance is 2e-2; measured pipeline error
~4.3e-3): halves HBM traffic vs fp32r (startup-critical) and enables the
fast-weight-load path on LDWEIGHTS. PSUM accumulation stays fp32.

Startup is the other big lever (the fixed engine preamble ends ~6.5us, and
the PE must be streaming real matmuls as soon after as possible): the
layer-0 weights and first seq chunk are spread across all four DMA queues
(sync/scalar HWDGE, gpsimd/vector SWDGE) so the first k-groups' operands
land ~2.5us after the rings go live, with a short burst of dummy bf16
matmuls keeping the PE-HAM clock gate warm until they do.
"""

import numpy as np

import ml_dtypes

import concourse.bass as bass
import concourse.tile as tile
from concourse import bacc, mybir
from concourse.bass_utils import run_bass_kernel_spmd

B, S, DIN, DH, DOUT = 8, 4096, 512, 1024, 512
SC = 512  # seq chunk processed per pipeline iteration

BF16 = mybir.dt.bfloat16
F32 = mybir.dt.float32

K0 = DIN // 128   # 4  k-tiles, layer 0
K1 = DH // 128    # 8  k-tiles, layers 1/2
M0 = DH // 128    # 8  m-tiles (feature tiles of h0/h1)
M2 = DOUT // 128  # 4  m-tiles (feature tiles of outT)
NCH = S // SC     # 8  chunks

N_WARM = 22      # dummy bf16 matmuls bridging preamble-end -> first data


def build_nc():
    nc = bacc.Bacc("TRN2")
    qT = nc.dram_tensor("qT", (DIN, S), BF16, kind="ExternalInput")
    w0t = nc.dram_tensor("w0t", (DIN, DH), BF16, kind="ExternalInput")
    w1t = nc.dram_tensor("w1t", (DH, DH), BF16, kind="ExternalInput")
    w2t = nc.dram_tensor("w2t", (DH, DOUT), BF16, kind="ExternalInput")
    # b0 | b1 | b2, host-packed as [128 partitions, 8+8+4 cols] so the load
    # is one contiguous-line DMA (the on-device scatter rearrange was taking
    # ~10us on the wire and stalling the ring's semaphore rotation).
    biases = nc.dram_tensor("biases", (128, 2 * M0 + M2), F32, kind="ExternalInput")
    outT = nc.dram_tensor("outT", (DOUT, S), F32, kind="ExternalOutput")

    Relu = mybir.ActivationFunctionType.Relu
    Ident = mybir.ActivationFunctionType.Identity

    with tile.TileContext(nc) as tc:
        with (
            tc.tile_pool(name="weights", bufs=1) as wpool,
            tc.tile_pool(name="biases", bufs=1) as bpool,
            tc.tile_pool(name="acts", bufs=2) as apool,
            tc.tile_pool(name="qin", bufs=3) as qpool,
            tc.tile_pool(name="outp", bufs=6) as opool,
            tc.tile_pool(name="psum0", bufs=3, space="PSUM") as ppool0,
            tc.tile_pool(name="psum1", bufs=3, space="PSUM") as ppool1,
            tc.tile_pool(name="psum2", bufs=2, space="PSUM") as ppool2,
        ):
            # Tiny warm tiles for the HAM warm-up matmuls.
            g_lhs = apool.tile([128, 128], BF16, tag="warm_lhs")
            g_rhs = apool.tile([128, 256], BF16, tag="warm_rhs")
            nc.vector.memset(g_lhs, 0.0)
            nc.vector.memset(g_rhs, 0.0)

            # ---- startup loads ----
            # The three DMA queues (sync/scalar HWDGE + gpsimd SWDGE) share
            # the ~358 GB/s HBM pipe (~110-120 GB/s each while all are
            # busy), and each is FIFO — so per-queue issue order is arranged
            # to make tiles land exactly in consumption order: the k-groups
            # of (w0, q0) first (w0 k-tiles split in half across both HWDGE
            # rings), then q1, then W1, then W2, then the q stream.
            w0_sb = [wpool.tile([128, DH], BF16, tag=f"w0_{k}", name=f"w0_{k}") for k in range(K0)]
            q0_sb = [qpool.tile([128, SC], BF16, tag=f"q_{k}", name=f"q0_{k}") for k in range(K0)]

            def packed_ktiles(t, row0, nk, ncols):
                # AP reading nk consecutive 128-row k-tiles of t, laid out
                # side by side on the 128 partitions: [p][k][col].
                base = t[row0:row0 + 128, 0:ncols]
                ap = [list(dim) for dim in base.ap]
                ap3 = [ap[0], [128 * ap[0][0], nk], ap[1]]
                return bass.AP(tensor=base.tensor, offset=base.offset, ap=ap3)

            def packed_qchunk(s0):
                base = qT[0:128, s0:s0 + SC]
                ap = [list(dim) for dim in base.ap]
                ap3 = [ap[0], [128 * ap[0][0], K0], ap[1]]
                return bass.AP(tensor=base.tensor, offset=base.offset, ap=ap3)

            # w0/q0 stay as per-k-tile pieces so chunk-0's k-outer layer 0
            # can start on k=0 while k=3 is still in flight. The scalar
            # engine gets ONLY these four small loads: any further issues
            # would block on the 4-deep DMA-semaphore rotation and stall
            # the relu/activation stream queued behind them.
            # q0[0] rides first on the (otherwise nearly empty) scalar
            # ring — its completion gates the very first real matmul.
            nc.scalar.dma_start(out=q0_sb[0], in_=qT[0:128, 0:SC])
            bias_sb = bpool.tile([128, 2 * M0 + M2], F32, tag="biases")

            for k in range(K0):
                if k > 0:
                    nc.gpsimd.dma_start(out=q0_sb[k], in_=qT[k * 128:(k + 1) * 128, 0:SC])
                nc.sync.dma_start(out=w0_sb[k][:, 0:DH // 2],
                                  in_=w0t[k * 128:(k + 1) * 128, 0:DH // 2])
                nc.scalar.dma_start(out=w0_sb[k][:, DH // 2:DH],
                                    in_=w0t[k * 128:(k + 1) * 128, DH // 2:DH])
                if k == 1:
                    nc.sync.dma_start(out=bias_sb, in_=biases[:, :])

            # Second q chunk as k-tile pieces, placed immediately after the
            # chunk-0-critical loads in each ring's FIFO (it is consumed
            # next).
            q1_sb = [qpool.tile([128, SC], BF16, tag=f"q_{k}", name=f"q1pre_{k}")
                     for k in range(K0)]
            nc.sync.dma_start(out=q1_sb[0], in_=qT[0:128, SC:2 * SC])
            nc.gpsimd.dma_start(out=q1_sb[1], in_=qT[128:256, SC:2 * SC])
            nc.gpsimd.dma_start(out=q1_sb[2], in_=qT[256:384, SC:2 * SC])
            nc.gpsimd.dma_start(out=q1_sb[3], in_=qT[384:512, SC:2 * SC])
            # W1 per-k-tile alternating sync/gpsimd so both rings finish
            # their share just before layer 1 of chunk 0 needs it (~23us);
            # W2 follows the same split. Nothing beyond w0 goes on scalar:
            # the Tile scheduler orders scalar's queue by consumer priority
            # and would push extra DMA issues behind the relu stream,
            # starving the ring.
            w1_sb = [wpool.tile([128, DH], BF16, tag=f"w1_{k}", name=f"w1_{k}") for k in range(K1)]
            w2_sb = [wpool.tile([128, DOUT], BF16, tag=f"w2_{k}", name=f"w2_{k}") for k in range(K1)]
            for k in range(K1):
                eng = nc.sync if k % 2 == 0 else nc.gpsimd
                eng.dma_start(out=w1_sb[k], in_=w1t[k * 128:(k + 1) * 128, :])
            for k in range(K1):
                eng = nc.sync if k % 2 == 0 else nc.gpsimd
                eng.dma_start(out=w2_sb[k], in_=w2t[k * 128:(k + 1) * 128, :])

            def w1_slice(k, m):
                return w1_sb[k][:, m * 128:(m + 1) * 128]

            def w2_slice(k, md):
                return w2_sb[k][:, md * 128:(md + 1) * 128]

            # ---- HAM warm-up: keep PE busy from preamble-end until the
            # first real operands land (dummy matmuls, garbage data).
            warm_ps = ppool0.tile([128, SC], F32, tag="ps0")
            for i in range(N_WARM):
                nc.tensor.matmul(
                    warm_ps[:, 0:256], lhsT=g_lhs, rhs=g_rhs,
                    start=(i == 0), stop=(i == N_WARM - 1),
                )

            def layer0_init(q_sb):
                """Chunk-0 layer 0, k-outer: all 8 m-tiles accumulate in
                parallel across all 8 PSUM banks, so the PE consumes
                (w0[k], q0[k]) in exactly the order the DMAs deliver them —
                no m-loop ever waits on a k-tile that hasn't landed."""
                pools = [ppool0, ppool0, ppool0, ppool1, ppool1, ppool1, ppool2, ppool2]
                tags = ["ps0", "ps0", "ps0", "ps1", "ps1", "ps1", "ps2", "ps2"]
                ps = [pools[m].tile([128, SC], F32, tag=tags[m], name=f"psI_{m}")
                      for m in range(M0)]
                h0_sb = []
                for k in range(K0):
                    for m in range(M0):
                        nc.tensor.matmul(
                            ps[m],
                            lhsT=w0_sb[k][:, m * 128:(m + 1) * 128],
                            rhs=q_sb[k],
                            start=(k == 0),
                            stop=(k == K0 - 1),
                        )
                        if k == K0 - 1:
                            h = apool.tile([128, SC], BF16, tag=f"h0_{m}",
                                           name=f"h0_0_{m}")
                            if m % 2 == 0:
                                nc.vector.tensor_scalar(
                                    h, ps[m], bias_sb[:, m:m + 1], 0.0,
                                    mybir.AluOpType.add, mybir.AluOpType.max)
                            else:
                                nc.scalar.activation(h, ps[m], Relu,
                                                     bias=bias_sb[:, m:m + 1])
                            h0_sb.append(h)
                return h0_sb

            def load_q(c):
                s0 = c * SC
                t = qpool.tile([128, K0 * SC], BF16, tag="qc", name=f"q{c}_all")
                nc.gpsimd.dma_start(out=t, in_=packed_qchunk(s0))
                return [t[:, k * SC:(k + 1) * SC] for k in range(K0)]

            def layer0(c, q_sb):
                h0_sb = []
                for m in range(M0):
                    ps = ppool0.tile([128, SC], F32, tag="ps0", name=f"ps0_{c}_{m}")
                    for k in range(K0):
                        nc.tensor.matmul(
                            ps,
                            lhsT=w0_sb[k][:, m * 128:(m + 1) * 128],
                            rhs=q_sb[k],
                            start=(k == 0),
                            stop=(k == K0 - 1),
                        )
                    h = apool.tile([128, SC], BF16, tag=f"h0_{m}", name=f"h0_{c}_{m}")
                    if m % 2 == 0:
                        nc.vector.tensor_scalar(
                            h, ps, bias_sb[:, m:m + 1], 0.0,
                            mybir.AluOpType.add, mybir.AluOpType.max)
                    else:
                        nc.scalar.activation(h, ps, Relu, bias=bias_sb[:, m:m + 1])
                    h0_sb.append(h)
                return h0_sb

            def layers12(c, h0_sb):
                s0 = c * SC
                h1_sb = []
                for m in range(M0):
                    ps = ppool1.tile([128, SC], F32, tag="ps1", name=f"ps1_{c}_{m}")
                    for k in range(K1):
                        nc.tensor.matmul(
                            ps,
                            lhsT=w1_slice(k, m),
                            rhs=h0_sb[k],
                            start=(k == 0),
                            stop=(k == K1 - 1),
                        )
                    h = apool.tile([128, SC], BF16, tag=f"h1_{m}", name=f"h1_{c}_{m}")
                    nc.scalar.activation(h, ps, Relu, bias=bias_sb[:, M0 + m:M0 + m + 1])
                    h1_sb.append(h)

                last = c == NCH - 1
                for md in range(M2):
                    ps = ppool2.tile([128, SC], F32, tag="ps2", name=f"ps2_{c}_{md}")
                    for k in range(K1):
                        nc.tensor.matmul(
                            ps,
                            lhsT=w2_slice(k, md),
                            rhs=h1_sb[k],
                            start=(k == 0),
                            stop=(k == K1 - 1),
                        )
                    if not (last and md == M2 - 1):
                        ot = opool.tile([128, SC], F32, tag="ot", name=f"ot_{c}_{md}")
                        nc.vector.tensor_scalar_add(
                            ot, ps, bias_sb[:, 2 * M0 + md:2 * M0 + md + 1])
                        nc.sync.dma_start(
                            out=outT[md * 128:(md + 1) * 128, s0:s0 + SC], in_=ot
                        )
                    else:
                        # Very last output tile: drain in two 256-col
                        # slices, second store on scalar (idle after its
                        # last ACT) — shortens the tail.
                        for sl in range(2):
                            o = opool.tile([128, SC // 2], F32, tag=f"otl_{sl}",
                                           name=f"otl_{c}_{md}_{sl}")
                            c0s = sl * (SC // 2)
                            nc.vector.tensor_scalar_add(
                                o, ps[:, c0s:c0s + SC // 2],
                                bias_sb[:, 2 * M0 + md:2 * M0 + md + 1],
                            )
                            eng = nc.sync if sl == 0 else nc.scalar
                            eng.dma_start(
                                out=outT[md * 128:(md + 1) * 128,
                                         s0 + c0s:s0 + c0s + SC // 2],
                                in_=o,
                            )

            # Software pipeline: emit L0 of chunk c+1 ahead of L1/L2 of
            # chunk c, so the matmul stream never depends on a DMA issued
            # less than a full chunk earlier.
            h0_cur = layer0_init(q0_sb)
            for c in range(NCH):
                h0_next = None
                if c + 1 < NCH:
                    q_sb = q1_sb if c + 1 == 1 else load_q(c + 1)
                    h0_next = layer0(c + 1, q_sb)
                layers12(c, h0_cur)
                h0_cur = h0_next
    nc.finalize()
    return nc


_NC = None


def _get_nc():
    global _NC
    if _NC is None:
        _NC = build_nc()
    return _NC


def make_in_maps(inputs):
    bf16 = ml_dtypes.bfloat16
    q, W0, b0, W1, b1, W2, b2 = (
        inputs["query"], inputs["W0"], inputs["b0"], inputs["W1"],
        inputs["b1"], inputs["W2"], inputs["b2"],
    )
    in_maps = []
    for b in range(B):
        in_maps.append({
            "qT": np.ascontiguousarray(np.asarray(q[b]).T).astype(bf16),
            "w0t": np.ascontiguousarray(np.asarray(W0[b]).T).astype(bf16),
            "w1t": np.ascontiguousarray(np.asarray(W1[b]).T).astype(bf16),
            "w2t": np.ascontiguousarray(np.asarray(W2[b]).T).astype(bf16),
            "biases": np.concatenate([
                np.asarray(b0[b], np.float32).reshape(M0, 128).T,
                np.asarray(b1[b], np.float32).reshape(M0, 128).T,
                np.asarray(b2[b], np.float32).reshape(M2, 128).T,
            ], axis=1).copy(),
        })
    return in_maps


def run(inputs, trace=False):
    nc = _get_nc()
    in_maps = make_in_maps(inputs)
    res = run_bass_kernel_spmd(nc, in_maps, core_ids=list(range(B)), trace=trace)
    out = np.stack(
        [np.asarray(r["outT"], dtype=np.float32).T for r in res.results]
    )
    return np.ascontiguousarray(out), res


def kernel(**inputs) -> np.ndarray:
    out, _ = run(inputs, trace=False)
    return out


# revision 19
# speedup vs baseline: 1.0188x; 1.0023x over previous
"""Trainium2 Bass kernel for nn_LongTermMemoryMLP.

Per-batch-weight 3-layer MLP:
    h0 = relu(q @ W0^T + b0); h1 = relu(h0 @ W1^T + b1); out = h1 @ W2^T + b2
with q: [B,S,DIN], W0: [B,DH,DIN], W1: [B,DH,DH], W2: [B,DOUT,DH], B=8.

Sharding: data-parallel over batch — one batch sample (and its weight slabs)
per NeuronCore, 8 cores, no cross-core communication.

Device-side strategy (v2): everything feature-major. Activations live as
[feature, seq] tiles (feature on partitions), weights are pre-transposed on
the host, so every layer is stationary=weight-slice, moving=activation —
including layer 2, whose output lands transposed ([DOUT, S]) and is
un-transposed on the host. That makes every bias a per-partition scalar,
applied for free in the scalar-engine activation that drains PSUM (no
broadcast-bias DMA, vector engine freed up for stores).

All matmul operands are bf16 (tolerance is 2e-2; measured pipeline error
~4.3e-3): halves HBM traffic vs fp32r (startup-critical) and enables the
fast-weight-load path on LDWEIGHTS. PSUM accumulation stays fp32.

Startup is the other big lever (the fixed engine preamble ends ~6.5us, and
the PE must be streaming real matmuls as soon after as possible): the
layer-0 weights and first seq chunk are spread across all four DMA queues
(sync/scalar HWDGE, gpsimd/vector SWDGE) so the first k-groups' operands
land ~2.5us after the rings go live, with a short burst of dummy bf16
matmuls keeping the PE-HAM clock gate warm until they do.
"""

import numpy as np

import ml_dtypes

import concourse.bass as bass
import concourse.tile as tile
from concourse import bacc, mybir
from concourse.bass_utils import run_bass_kernel_spmd

B, S, DIN, DH, DOUT = 8, 4096, 512, 1024, 512
SC = 512  # seq chunk processed per pipeline iteration

BF16 = mybir.dt.bfloat16
F32 = mybir.dt.float32

K0 = DIN // 128   # 4  k-tiles, layer 0
K1 = DH // 128    # 8  k-tiles, layers 1/2
M0 = DH // 128    # 8  m-tiles (feature tiles of h0/h1)
M2 = DOUT // 128  # 4  m-tiles (feature tiles of outT)
NCH = S // SC     # 8  chunks

N_WARM = 22      # dummy bf16 matmuls bridging preamble-end -> first data


def build_nc():
    nc = bacc.Bacc("TRN2")
    qT = nc.dram_tensor("qT", (DIN, S), BF16, kind="ExternalInput")
    w0t = nc.dram_tensor("w0t", (DIN, DH), BF16, kind="ExternalInput")
    w1t = nc.dram_tensor("w1t", (DH, DH), BF16, kind="ExternalInput")
    w2t = nc.dram_tensor("w2t", (DH, DOUT), BF16, kind="ExternalInput")
    # b0 | b1 | b2, host-packed as [128 partitions, 8+8+4 cols] so the load
    # is one contiguous-line DMA (the on-device scatter rearrange was taking
    # ~10us on the wire and stalling the ring's semaphore rotation).
    biases = nc.dram_tensor("biases", (128, 2 * M0 + M2), F32, kind="ExternalInput")
    # Output stored bf16 (host upcasts to f32): halves store traffic,
    # doubles the DVE drain rate, shrinks the kernel tail. Adds ~0.22%
    # quantization error on top of ~0.43% from the bf16 matmuls.
    outT = nc.dram_tensor("outT", (DOUT, S), BF16, kind="ExternalOutput")

    Relu = mybir.ActivationFunctionType.Relu
    Ident = mybir.ActivationFunctionType.Identity

    with tile.TileContext(nc) as tc:
        with (
            tc.tile_pool(name="weights", bufs=1) as wpool,
            tc.tile_pool(name="biases", bufs=1) as bpool,
            tc.tile_pool(name="acts", bufs=2) as apool,
            tc.tile_pool(name="qin", bufs=3) as qpool,
            tc.tile_pool(name="outp", bufs=6) as opool,
            tc.tile_pool(name="psum0", bufs=3, space="PSUM") as ppool0,
            tc.tile_pool(name="psum1", bufs=3, space="PSUM") as ppool1,
            tc.tile_pool(name="psum2", bufs=2, space="PSUM") as ppool2,
        ):
            # Tiny warm tiles for the HAM warm-up matmuls.
            g_lhs = apool.tile([128, 128], BF16, tag="warm_lhs")
            g_rhs = apool.tile([128, 256], BF16, tag="warm_rhs")
            nc.vector.memset(g_lhs, 0.0)
            nc.vector.memset(g_rhs, 0.0)

            # ---- startup loads ----
            # The three DMA queues (sync/scalar HWDGE + gpsimd SWDGE) share
            # the ~358 GB/s HBM pipe (~110-120 GB/s each while all are
            # busy), and each is FIFO — so per-queue issue order is arranged
            # to make tiles land exactly in consumption order: the k-groups
            # of (w0, q0) first (w0 k-tiles split in half across both HWDGE
            # rings), then q1, then W1, then W2, then the q stream.
            w0_sb = [wpool.tile([128, DH], BF16, tag=f"w0_{k}", name=f"w0_{k}") for k in range(K0)]
            q0_sb = [qpool.tile([128, SC], BF16, tag=f"q_{k}", name=f"q0_{k}") for k in range(K0)]

            def packed_ktiles(t, row0, nk, ncols):
                # AP reading nk consecutive 128-row k-tiles of t, laid out
                # side by side on the 128 partitions: [p][k][col].
                base = t[row0:row0 + 128, 0:ncols]
                ap = [list(dim) for dim in base.ap]
                ap3 = [ap[0], [128 * ap[0][0], nk], ap[1]]
                return bass.AP(tensor=base.tensor, offset=base.offset, ap=ap3)

            def packed_qchunk(s0):
                base = qT[0:128, s0:s0 + SC]
                ap = [list(dim) for dim in base.ap]
                ap3 = [ap[0], [128 * ap[0][0], K0], ap[1]]
                return bass.AP(tensor=base.tensor, offset=base.offset, ap=ap3)

            # w0/q0 stay as per-k-tile pieces so chunk-0's k-outer layer 0
            # can start on k=0 while k=3 is still in flight. The scalar
            # engine gets ONLY these four small loads: any further issues
            # would block on the 4-deep DMA-semaphore rotation and stall
            # the relu/activation stream queued behind them.
            # q0[0] rides first on the (otherwise nearly empty) scalar
            # ring — its completion gates the very first real matmul.
            nc.scalar.dma_start(out=q0_sb[0], in_=qT[0:128, 0:SC])
            bias_sb = bpool.tile([128, 2 * M0 + M2], F32, tag="biases")

            for k in range(K0):
                if k > 0:
                    nc.gpsimd.dma_start(out=q0_sb[k], in_=qT[k * 128:(k + 1) * 128, 0:SC])
                nc.sync.dma_start(out=w0_sb[k][:, 0:DH // 2],
                                  in_=w0t[k * 128:(k + 1) * 128, 0:DH // 2])
                nc.scalar.dma_start(out=w0_sb[k][:, DH // 2:DH],
                                    in_=w0t[k * 128:(k + 1) * 128, DH // 2:DH])
                if k == 1:
                    nc.sync.dma_start(out=bias_sb, in_=biases[:, :])

            # Second q chunk as k-tile pieces, placed immediately after the
            # chunk-0-critical loads in each ring's FIFO (it is consumed
            # next).
            q1_sb = [qpool.tile([128, SC], BF16, tag=f"q_{k}", name=f"q1pre_{k}")
                     for k in range(K0)]
            nc.sync.dma_start(out=q1_sb[0], in_=qT[0:128, SC:2 * SC])
            nc.gpsimd.dma_start(out=q1_sb[1], in_=qT[128:256, SC:2 * SC])
            nc.gpsimd.dma_start(out=q1_sb[2], in_=qT[256:384, SC:2 * SC])
            nc.gpsimd.dma_start(out=q1_sb[3], in_=qT[384:512, SC:2 * SC])
            # W1 per-k-tile alternating sync/gpsimd so both rings finish
            # their share just before layer 1 of chunk 0 needs it (~23us);
            # W2 follows the same split. Nothing beyond w0 goes on scalar:
            # the Tile scheduler orders scalar's queue by consumer priority
            # and would push extra DMA issues behind the relu stream,
            # starving the ring.
            w1_sb = [wpool.tile([128, DH], BF16, tag=f"w1_{k}", name=f"w1_{k}") for k in range(K1)]
            w2_sb = [wpool.tile([128, DOUT], BF16, tag=f"w2_{k}", name=f"w2_{k}") for k in range(K1)]
            for k in range(K1):
                eng = nc.sync if k % 2 == 0 else nc.gpsimd
                eng.dma_start(out=w1_sb[k], in_=w1t[k * 128:(k + 1) * 128, :])
            for k in range(K1):
                eng = nc.sync if k % 2 == 0 else nc.gpsimd
                eng.dma_start(out=w2_sb[k], in_=w2t[k * 128:(k + 1) * 128, :])

            def w1_slice(k, m):
                return w1_sb[k][:, m * 128:(m + 1) * 128]

            def w2_slice(k, md):
                return w2_sb[k][:, md * 128:(md + 1) * 128]

            # ---- HAM warm-up: keep PE busy from preamble-end until the
            # first real operands land (dummy matmuls, garbage data).
            warm_ps = ppool0.tile([128, SC], F32, tag="ps0")
            for i in range(N_WARM):
                nc.tensor.matmul(
                    warm_ps[:, 0:256], lhsT=g_lhs, rhs=g_rhs,
                    start=(i == 0), stop=(i == N_WARM - 1),
                )

            def layer0_init(q_sb):
                """Chunk-0 layer 0, k-outer: all 8 m-tiles accumulate in
                parallel across all 8 PSUM banks, so the PE consumes
                (w0[k], q0[k]) in exactly the order the DMAs deliver them —
                no m-loop ever waits on a k-tile that hasn't landed."""
                pools = [ppool0, ppool0, ppool0, ppool1, ppool1, ppool1, ppool2, ppool2]
                tags = ["ps0", "ps0", "ps0", "ps1", "ps1", "ps1", "ps2", "ps2"]
                ps = [pools[m].tile([128, SC], F32, tag=tags[m], name=f"psI_{m}")
                      for m in range(M0)]
                h0_sb = []
                for k in range(K0):
                    for m in range(M0):
                        nc.tensor.matmul(
                            ps[m],
                            lhsT=w0_sb[k][:, m * 128:(m + 1) * 128],
                            rhs=q_sb[k],
                            start=(k == 0),
                            stop=(k == K0 - 1),
                        )
                        if k == K0 - 1:
                            h = apool.tile([128, SC], BF16, tag=f"h0_{m}",
                                           name=f"h0_0_{m}")
                            if m % 2 == 0:
                                nc.vector.tensor_scalar(
                                    h, ps[m], bias_sb[:, m:m + 1], 0.0,
                                    mybir.AluOpType.add, mybir.AluOpType.max)
                            else:
                                nc.scalar.activation(h, ps[m], Relu,
                                                     bias=bias_sb[:, m:m + 1])
                            h0_sb.append(h)
                return h0_sb

            def load_q(c):
                s0 = c * SC
                t = qpool.tile([128, K0 * SC], BF16, tag="qc", name=f"q{c}_all")
                nc.gpsimd.dma_start(out=t, in_=packed_qchunk(s0))
                return [t[:, k * SC:(k + 1) * SC] for k in range(K0)]

            def layer0(c, q_sb):
                h0_sb = []
                for m in range(M0):
                    ps = ppool0.tile([128, SC], F32, tag="ps0", name=f"ps0_{c}_{m}")
                    for k in range(K0):
                        nc.tensor.matmul(
                            ps,
                            lhsT=w0_sb[k][:, m * 128:(m + 1) * 128],
                            rhs=q_sb[k],
                            start=(k == 0),
                            stop=(k == K0 - 1),
                        )
                    h = apool.tile([128, SC], BF16, tag=f"h0_{m}", name=f"h0_{c}_{m}")
                    if m % 2 == 0:
                        nc.vector.tensor_scalar(
                            h, ps, bias_sb[:, m:m + 1], 0.0,
                            mybir.AluOpType.add, mybir.AluOpType.max)
                    else:
                        nc.scalar.activation(h, ps, Relu, bias=bias_sb[:, m:m + 1])
                    h0_sb.append(h)
                return h0_sb

            def layers12(c, h0_sb):
                s0 = c * SC
                h1_sb = []
                for m in range(M0):
                    ps = ppool1.tile([128, SC], F32, tag="ps1", name=f"ps1_{c}_{m}")
                    for k in range(K1):
                        nc.tensor.matmul(
                            ps,
                            lhsT=w1_slice(k, m),
                            rhs=h0_sb[k],
                            start=(k == 0),
                            stop=(k == K1 - 1),
                        )
                    h = apool.tile([128, SC], BF16, tag=f"h1_{m}", name=f"h1_{c}_{m}")
                    nc.scalar.activation(h, ps, Relu, bias=bias_sb[:, M0 + m:M0 + m + 1])
                    h1_sb.append(h)

                last = c == NCH - 1
                for md in range(M2):
                    ps = ppool2.tile([128, SC], F32, tag="ps2", name=f"ps2_{c}_{md}")
                    for k in range(K1):
                        nc.tensor.matmul(
                            ps,
                            lhsT=w2_slice(k, md),
                            rhs=h1_sb[k],
                            start=(k == 0),
                            stop=(k == K1 - 1),
                        )
                    if not (last and md == M2 - 1):
                        ot = opool.tile([128, SC], BF16, tag="ot", name=f"ot_{c}_{md}")
                        nc.vector.tensor_scalar_add(
                            ot, ps, bias_sb[:, 2 * M0 + md:2 * M0 + md + 1])
                        nc.sync.dma_start(
                            out=outT[md * 128:(md + 1) * 128, s0:s0 + SC], in_=ot
                        )
                    else:
                        # Very last output tile: drain in two 256-col
                        # slices, second store on scalar (idle after its
                        # last ACT) — shortens the tail.
                        for sl in range(2):
                            o = opool.tile([128, SC // 2], BF16, tag=f"otl_{sl}",
                                           name=f"otl_{c}_{md}_{sl}")
                            c0s = sl * (SC // 2)
                            nc.vector.tensor_scalar_add(
                                o, ps[:, c0s:c0s + SC // 2],
                                bias_sb[:, 2 * M0 + md:2 * M0 + md + 1],
                            )
                            eng = nc.sync if sl == 0 else nc.scalar
                            eng.dma_start(
                                out=outT[md * 128:(md + 1) * 128,
                                         s0 + c0s:s0 + c0s + SC // 2],
                                in_=o,
                            )

            # Software pipeline: emit L0 of chunk c+1 ahead of L1/L2 of
            # chunk c, so the matmul stream never depends on a DMA issued
            # less than a full chunk earlier.
            h0_cur = layer0_init(q0_sb)
            for c in range(NCH):
                h0_next = None
                if c + 1 < NCH:
                    q_sb = q1_sb if c + 1 == 1 else load_q(c + 1)
                    h0_next = layer0(c + 1, q_sb)
                layers12(c, h0_cur)
                h0_cur = h0_next
    nc.finalize()
    return nc


_NC = None


def _get_nc():
    global _NC
    if _NC is None:
        _NC = build_nc()
    return _NC


def make_in_maps(inputs):
    bf16 = ml_dtypes.bfloat16
    q, W0, b0, W1, b1, W2, b2 = (
        inputs["query"], inputs["W0"], inputs["b0"], inputs["W1"],
        inputs["b1"], inputs["W2"], inputs["b2"],
    )
    in_maps = []
    for b in range(B):
        in_maps.append({
            "qT": np.ascontiguousarray(np.asarray(q[b]).T).astype(bf16),
            "w0t": np.ascontiguousarray(np.asarray(W0[b]).T).astype(bf16),
            "w1t": np.ascontiguousarray(np.asarray(W1[b]).T).astype(bf16),
            "w2t": np.ascontiguousarray(np.asarray(W2[b]).T).astype(bf16),
            "biases": np.concatenate([
                np.asarray(b0[b], np.float32).reshape(M0, 128).T,
                np.asarray(b1[b], np.float32).reshape(M0, 128).T,
                np.asarray(b2[b], np.float32).reshape(M2, 128).T,
            ], axis=1).copy(),
        })
    return in_maps


def run(inputs, trace=False):
    nc = _get_nc()
    in_maps = make_in_maps(inputs)
    res = run_bass_kernel_spmd(nc, in_maps, core_ids=list(range(B)), trace=trace)
    out = np.stack(
        [np.asarray(r["outT"]).astype(np.float32).T for r in res.results]
    )
    return np.ascontiguousarray(out), res


def kernel(**inputs) -> np.ndarray:
    out, _ = run(inputs, trace=False)
    return out
